# revision 19
# baseline (speedup 1.0000x reference)
"""PTQLinear (smoothquant int8 PTQ linear) on 8 Trainium2 NeuronCores.

Sharding: data-parallel over M for x (M/8 rows/core), over N for the
weight-quantization work (N/8 rows/core), over rows for calibration.
Two small AllReduce-max collectives carry the per-channel amax partials
(calibration+weight first — they gate weight quant; x column-max second
— it gates only the input scale); the quantized weight (bf16-encoded
int8) and per-row weight scales are AllGathered.  The int8 GEMM runs as
bf16 matmuls on the PE (integers <= 127 are exact in bf16; products and
partial sums are exact in fp32 PSUM), so the int32 accumulator matches
the reference bit-for-bit.

Engine routing: big loads/stores on SP-HWDGE, broadcast loads + weight
chunk streaming on ACT-HWDGE, |x| via ACT activation, transposes on the
(otherwise idle) PE via identity matmul with ACT copying PSUM->SBUF,
quant arithmetic on DVE (exact RNE rounding via the +-1.5*2^23 trick).
"""

from contextlib import ExitStack

import numpy as np

import concourse.bass as bass
import concourse.tile as tile
from concourse import bacc, mybir
from concourse.bass_utils import run_bass_kernel_spmd
from concourse.masks import make_identity

F32 = mybir.dt.float32
BF16 = mybir.dt.bfloat16
AX = mybir.AxisListType
OP = mybir.AluOpType
ACTF = mybir.ActivationFunctionType

MAGIC = 12582912.0  # 1.5 * 2**23: RNE round-to-int for |v| << 2**22
R127 = float(np.float32(1.0) / np.float32(127.0))


def _fold_partitions_pe(nc, psum, identf, part, res2d, KT):
    """Cross-partition max of a [128, K] f32 tile (values >= 0) via PE
    transposes of 128x128 blocks + DVE free-dim reduces.  Result layout:
    res2d[p, b] = colmax of channel 128*b + p (f-major)."""
    for b in range(KT):
        tps = psum.tile([128, 512], F32, tag="tps", bufs=4)
        nc.tensor.transpose(tps[:, 0:128], part[:, 128 * b : 128 * (b + 1)], identf[:])
        nc.vector.tensor_reduce(res2d[:, b : b + 1], tps[:, 0:128], axis=AX.X, op=OP.max)


def _sqrt_refined(nc, pool, a, out, P, F, iters=2):
    """out = sqrt(a) for [P, F] f32 tiles, ACT seed + Newton via DVE."""
    nc.scalar.activation(out[:], a[:], ACTF.Sqrt)
    for _ in range(iters):
        r = pool.tile([P, F], F32, tag="sqr_r")
        h = pool.tile([P, F], F32, tag="sqr_h")
        nc.vector.reciprocal(r[:], out[:])
        nc.vector.tensor_tensor(h[:], a[:], r[:], op=OP.mult)  # ~ a / y
        nc.vector.tensor_tensor(out[:], out[:], h[:], op=OP.add)
        nc.vector.tensor_scalar(out[:], out[:], 0.5, None, op0=OP.mult)


def _recip_refined(nc, pool, a, out, P, F):
    """out = 1/a (f32), InstReciprocal + one Newton step."""
    r0 = pool.tile([P, F], F32, tag="rcp_r0")
    u = pool.tile([P, F], F32, tag="rcp_u")
    t = pool.tile([P, F], F32, tag="rcp_t")
    nc.vector.reciprocal(r0[:], a[:])
    nc.vector.tensor_tensor(u[:], a[:], r0[:], op=OP.mult)
    nc.vector.tensor_tensor(t[:], r0[:], u[:], op=OP.mult)
    # out = 2*r0 - r0*u
    nc.vector.scalar_tensor_tensor(out[:], r0[:], 2.0, t[:], op0=OP.mult, op1=OP.subtract)


def _div127(nc, pool, num, out, P, F):
    """out = correctly-rounded num / 127 (Newton residual correction)."""
    q0 = pool.tile([P, F], F32, tag="divq0")
    e = pool.tile([P, F], F32, tag="dive")
    nc.vector.tensor_scalar(q0[:], num[:], R127, None, op0=OP.mult)
    nc.vector.scalar_tensor_tensor(e[:], q0[:], -127.0, num[:], op0=OP.mult, op1=OP.add)
    nc.vector.scalar_tensor_tensor(out[:], e[:], R127, q0[:], op0=OP.mult, op1=OP.add)


def build_bass(M, K, N, CAL, n_cores):
    """Build the per-core SPMD Bass module (all cores run the same program)."""
    C = n_cores
    MC, NC, CALC = M // C, N // C, CAL // C
    MT, NWT, CT, KT = MC // 128, NC // 128, CALC // 128, K // 128
    KP, NP = K // 128, N // 128
    NCH = min(512, NC)          # matmul n-chunk (free dim)
    NCHT = NC // NCH            # chunks per rank slice
    assert MC % 128 == 0 and NC % 128 == 0 and CALC % 128 == 0 and K % 128 == 0

    nc = bacc.Bacc(None, num_devices=C)
    groups = [list(range(C))]

    x_h = nc.dram_tensor("x", [MC, K], F32, kind="ExternalInput")
    w_h = nc.dram_tensor("w", [NC, K], F32, kind="ExternalInput")
    cal_h = nc.dram_tensor("cal", [CALC, K], F32, kind="ExternalInput")
    bias_h = nc.dram_tensor("bias", [N], F32, kind="ExternalInput")
    out_h = nc.dram_tensor("out", [MC, N], F32, kind="ExternalOutput")

    with tile.TileContext(nc) as tc:
        with ExitStack() as octx:
            dram = octx.enter_context(tc.tile_pool(name="dram", bufs=1, space="DRAM"))
            smalls = octx.enter_context(tc.tile_pool(name="smalls", bufs=1))
            psum = octx.enter_context(tc.tile_pool(name="psum", bufs=1, space="PSUM"))

            # internal DRAM
            cc_a_in = dram.tile([2, 128, KP], F32)
            cc_a_out = dram.tile([2, 128, KP], F32, addr_space="Shared")
            cc_b_in = dram.tile([128, KP], F32)
            cc_b_out = dram.tile([128, KP], F32, addr_space="Shared")
            wq_mine_d = dram.tile([K, NC], mybir.dt.int8)
            wq_all_d = dram.tile([C, K, NC], mybir.dt.int8, addr_space="Shared")
            ws_mine_d = dram.tile([NC], F32)
            ws_all_d = dram.tile([C, NC], F32, addr_space="Shared")
            smooth_d = dram.tile([K], F32)
            c_d = dram.tile([K], F32)
            pv_d = dram.tile([N], F32)

            ident = smalls.tile([128, 128], BF16, tag="ident")
            make_identity(nc, ident[:])
            identf = smalls.tile([128, 128], F32, tag="identf")
            make_identity(nc, identf[:])

            # ---- Phase A1: cal + weight per-channel abs-max partials --------
            def acc_one(apool, part, src_h, i, first, tag):
                t = apool.tile([128, K], F32, tag=tag)
                eng = nc.sync if i % 2 == 0 else nc.scalar
                eng.dma_start(t[:], src_h[128 * i : 128 * (i + 1), :])
                a = apool.tile([128, K], F32, tag="abs_tmp")
                nc.scalar.activation(a[:], t[:], ACTF.Abs)
                if first:
                    nc.vector.tensor_copy(part[:], a[:])
                else:
                    nc.vector.tensor_tensor(part[:], part[:], a[:], op=OP.max)

            def acc_absmax(apool, part, src_h, count, tag):
                for i in range(count):
                    acc_one(apool, part, src_h, i, i == 0, tag)

            actx = ExitStack()
            apool = actx.enter_context(tc.tile_pool(name="apool", bufs=2))
            xcol_part = None
            with tc.tile_pool(name="parts", bufs=1) as parts:
                cal_part = parts.tile([128, K], F32, tag="cal_part")
                w_part = parts.tile([128, K], F32, tag="w_part")

                acc_absmax(apool, cal_part, cal_h, CT, "ld_t")
                acc_absmax(apool, w_part, w_h, NWT, "ld_t")
                cal2d = smalls.tile([128, KP], F32, tag="cal2d")
                w2d = smalls.tile([128, KP], F32, tag="w2d")
                _fold_partitions_pe(nc, psum, identf, cal_part, cal2d, KT)
                _fold_partitions_pe(nc, psum, identf, w_part, w2d, KT)
                nc.sync.dma_start(cc_a_in[0], cal2d[:])
                nc.scalar.dma_start(cc_a_in[1], w2d[:])
                nc.gpsimd.collective_compute(
                    "AllReduce", OP.max, replica_groups=groups,
                    ins=[cc_a_in[:]], outs=[cc_a_out[:]],
                )
            parts_x = actx.enter_context(tc.tile_pool(name="parts_x", bufs=1))
            xcol_part = parts_x.tile([128, K], F32, tag="xcol_part")

            # ---- Phase B1: smooth / input_transform (after CC1a) ------------
            act_t = smalls.tile([128, KP], F32, tag="act_t")
            wcs_t = smalls.tile([128, KP], F32, tag="wcs_t")
            nc.sync.dma_start(act_t[:], cc_a_out[0])
            nc.sync.dma_start(wcs_t[:], cc_a_out[1])
            nc.vector.tensor_scalar(act_t[:], act_t[:], 1e-4, None, op0=OP.max)
            nc.vector.tensor_scalar(wcs_t[:], wcs_t[:], 1e-4, None, op0=OP.max)

            sa = smalls.tile([128, KP], F32, tag="sa")
            sw = smalls.tile([128, KP], F32, tag="sw")
            _sqrt_refined(nc, smalls, act_t, sa, 128, KP)
            _sqrt_refined(nc, smalls, wcs_t, sw, 128, KP)
            rsw = smalls.tile([128, KP], F32, tag="rsw")
            _recip_refined(nc, smalls, sw, rsw, 128, KP)
            smooth = smalls.tile([128, KP], F32, tag="smooth")
            nc.vector.tensor_tensor(smooth[:], sa[:], rsw[:], op=OP.mult)
            nc.vector.tensor_scalar(smooth[:], smooth[:], 4.0, 0.25, op0=OP.min, op1=OP.max)
            it2d = smalls.tile([128, KP], F32, tag="it2d")
            _recip_refined(nc, smalls, smooth, it2d, 128, KP)
            nc.sync.dma_start(smooth_d[:].rearrange("(f p) -> p f", p=128), smooth[:])

            # ---- Phase C: weight quant + PE transpose + AllGather -----------
            with tc.tile_pool(name="cpool", bufs=1) as cpool:
                smooth_bc = cpool.tile([128, K], F32, tag="smooth_bc")
                wqt = cpool.tile([128, KT, NC], BF16, tag="wqt")
                wq8 = cpool.tile([128, KT, NC], mybir.dt.int8, tag="wq8")
                nc.scalar.dma_start(
                    smooth_bc[:],
                    smooth_d[:].rearrange("(a k) -> a k", a=1).broadcast_to([128, K]),
                )
                with tc.tile_pool(name="wpool2", bufs=2) as wpool2, \
                     tc.tile_pool(name="wqpool", bufs=2) as wqpool:
                    for i in range(NWT):
                        wt = wpool2.tile([128, K], F32, tag="w_t2")
                        weng = nc.sync if i % 2 == 0 else nc.scalar
                        weng.dma_start(wt[:], w_h[128 * i : 128 * (i + 1), :])
                        nc.vector.tensor_tensor(wt[:], wt[:], smooth_bc[:], op=OP.mult)
                        ws_raw = smalls.tile([128, 1], F32, tag="ws_raw")
                        nc.vector.tensor_reduce(ws_raw[:], wt[:], axis=AX.X, op=OP.max,
                                                apply_absolute_value=True)
                        ws = smalls.tile([128, 1], F32, tag="ws")
                        _div127(nc, smalls, ws_raw, ws, 128, 1)
                        nc.vector.tensor_scalar(ws[:], ws[:], 1e-8, None, op0=OP.max)
                        rws = smalls.tile([128, 1], F32, tag="rws")
                        _recip_refined(nc, smalls, ws, rws, 128, 1)
                        # q0 = tw * (1/ws) on ACT, round + clip on DVE, cast bf16
                        nc.scalar.activation(wt[:], wt[:], ACTF.Copy, scale=rws[:])
                        wq = wqpool.tile([128, K], BF16, tag="wq")
                        nc.vector.tensor_scalar(wq[:], wt[:], MAGIC, MAGIC,
                                                op0=OP.add, op1=OP.subtract)
                        for g in range(KT // 4):
                            tps = psum.tile([128, 512], BF16, tag="tps", bufs=4)
                            for q in range(4):
                                k = 4 * g + q
                                nc.tensor.transpose(
                                    tps[:, 128 * q : 128 * (q + 1)],
                                    wq[:, 128 * k : 128 * (k + 1)], ident[:])
                            dst = wqt[:, 4 * g : 4 * g + 4, 128 * i : 128 * (i + 1)]
                            srcv = tps[:].rearrange("p (a b) -> p a b", a=4)
                            if g % 2 == 0:
                                nc.vector.tensor_copy(dst, srcv)
                            else:
                                nc.scalar.copy(dst, srcv)
                            if i == NWT - 1:
                                # all w-tiles done for these k-slices: cast to
                                # int8 + store for the AllGather immediately
                                for q in range(4):
                                    k = 4 * g + q
                                    if k % 2 == 0:
                                        nc.vector.tensor_copy(wq8[:, k, :], wqt[:, k, :])
                                    else:
                                        nc.scalar.copy(wq8[:, k, :], wqt[:, k, :])
                                    seng = nc.sync if k % 2 == 0 else nc.scalar
                                    seng.dma_start(
                                        wq_mine_d[128 * k : 128 * (k + 1), :],
                                        wq8[:, k, :])
                        nc.sync.dma_start(
                            ws_mine_d[128 * i : 128 * (i + 1)]
                            .rearrange("(p f) -> p f", p=128),
                            ws[:],
                        )
                        # interleave x column-max partial tiles so they run in
                        # DVE gaps without delaying the AllGather
                        for j in range(i * MT // NWT, (i + 1) * MT // NWT):
                            acc_one(apool, xcol_part, x_h, j, j == 0, "ld_t")
            nc.gpsimd.collective_compute(
                "AllGather", OP.bypass, replica_groups=groups,
                ins=[wq_mine_d[:]], outs=[wq_all_d[:]],
            )
            nc.gpsimd.collective_compute(
                "AllGather", OP.bypass, replica_groups=groups,
                ins=[ws_mine_d[:]], outs=[ws_all_d[:]],
            )

            # ---- Phase A2 tail: fold x colmax + CC1b ------------------------
            xcol2d = smalls.tile([128, KP], F32, tag="xcol2d")
            _fold_partitions_pe(nc, psum, identf, xcol_part, xcol2d, KT)
            nc.sync.dma_start(cc_b_in[:], xcol2d[:])
            nc.gpsimd.collective_compute(
                "AllReduce", OP.max, replica_groups=groups,
                ins=[cc_b_in[:]], outs=[cc_b_out[:]],
            )
            actx.close()

            # ---- Phase B2: input scale s and combined quant factor ----------
            xcol_t = smalls.tile([128, KP], F32, tag="xcol_t")
            nc.sync.dma_start(xcol_t[:], cc_b_out[:])
            am_t = smalls.tile([128, KP], F32, tag="am_t")
            nc.vector.tensor_tensor(am_t[:], xcol_t[:], it2d[:], op=OP.mult)
            am_col = smalls.tile([128, 1], F32, tag="am_col")
            nc.vector.tensor_reduce(am_col[:], am_t[:], axis=AX.X, op=OP.max,
                                    apply_absolute_value=True)
            am_row = smalls.tile([1, 128], F32, tag="am_row")
            nc.sync.dma_start(am_row[:], am_col[:])
            amax = smalls.tile([1, 1], F32, tag="amax")
            nc.vector.tensor_reduce(amax[:], am_row[:], axis=AX.X, op=OP.max)

            s_t = smalls.tile([1, 1], F32, tag="s_t")
            _div127(nc, smalls, amax, s_t, 1, 1)
            nc.vector.tensor_scalar(s_t[:], s_t[:], 1e-8, None, op0=OP.max)
            rs_t = smalls.tile([1, 1], F32, tag="rs_t")
            _recip_refined(nc, smalls, s_t, rs_t, 1, 1)
            # rs broadcast to [128, 1] so it can scale it2d per-partition
            rs_d = dram.tile([1, 1], F32)
            nc.sync.dma_start(rs_d[:], rs_t[:])
            rs_bc = smalls.tile([128, 1], F32, tag="rs_bc")
            nc.sync.dma_start(rs_bc[:], rs_d[:].broadcast_to([128, 1]))
            # combined per-channel factor c = input_transform * (1/s)
            c2d = smalls.tile([128, KP], F32, tag="c2d")
            nc.vector.tensor_scalar(c2d[:], it2d[:], rs_bc[:], None, op0=OP.mult)
            nc.sync.dma_start(c_d[:].rearrange("(f p) -> p f", p=128), c2d[:])

            # ---- Phase D: x quantization + PE transpose ---------------------
            with tc.tile_pool(name="p_xqt", bufs=1) as p_xqt:
                xqt = p_xqt.tile([128, KT, MC], BF16, tag="xqt")
                with tc.tile_pool(name="p_cbc", bufs=1) as p_cbc, \
                     tc.tile_pool(name="xpool2", bufs=2) as xpool2, \
                     tc.tile_pool(name="xqpool", bufs=3) as xqpool:
                    c_bc = p_cbc.tile([128, K], F32, tag="c_bc")
                    nc.scalar.dma_start(
                        c_bc[:],
                        c_d[:].rearrange("(a k) -> a k", a=1).broadcast_to([128, K]),
                    )
                    for i in range(MT):
                        xt = xpool2.tile([128, K], F32, tag="x_t2")
                        xeng = nc.sync if i % 2 == 0 else nc.scalar
                        xeng.dma_start(xt[:], x_h[128 * i : 128 * (i + 1), :])
                        nc.vector.tensor_tensor(xt[:], xt[:], c_bc[:], op=OP.mult)
                        xq = xqpool.tile([128, K], BF16, tag="xq")
                        # |tx/s| <= 127 by construction, so no clip needed;
                        # the bf16 cast on write is exact for small ints.
                        nc.vector.tensor_scalar(xq[:], xt[:], MAGIC, MAGIC,
                                                op0=OP.add, op1=OP.subtract)
                        for g in range(KT // 4):
                            tps = psum.tile([128, 512], BF16, tag="tps", bufs=4)
                            for q in range(4):
                                k = 4 * g + q
                                nc.tensor.transpose(
                                    tps[:, 128 * q : 128 * (q + 1)],
                                    xq[:, 128 * k : 128 * (k + 1)], ident[:])
                            dst = xqt[:, 4 * g : 4 * g + 4, 128 * i : 128 * (i + 1)]
                            srcv = tps[:].rearrange("p (a b) -> p a b", a=4)
                            if g % 2 == 0:
                                nc.vector.tensor_copy(dst, srcv)
                            else:
                                nc.scalar.copy(dst, srcv)

                # ---- pv = input_scale * weight_scale [N] + bias (emitted
                # after phase D so its gather-dependent DMAs don't block the
                # x-quant queue) ------------------------------------------
                with tc.tile_pool(name="p_pvb", bufs=1) as p_pvb:
                    ws2d = smalls.tile([128, NP], F32, tag="ws2d")
                    nc.scalar.dma_start(
                        ws2d[:], ws_all_d[:].rearrange("c (pc f) -> (c pc) f", f=NP)
                    )
                    s_bcd = dram.tile([1, 1], F32)
                    nc.sync.dma_start(s_bcd[:], s_t[:])
                    s_bc = smalls.tile([128, 1], F32, tag="s_bc")
                    nc.scalar.dma_start(s_bc[:], s_bcd[:].broadcast_to([128, 1]))
                    pv2d = smalls.tile([128, NP], F32, tag="pv2d")
                    nc.vector.tensor_scalar(pv2d[:], ws2d[:], s_bc[:], None, op0=OP.mult)
                    nc.sync.dma_start(pv_d[:].rearrange("(p f) -> p f", p=128), pv2d[:])
                    pv_bc = p_pvb.tile([128, N], F32, tag="pv_bc")
                    bias_bc = p_pvb.tile([128, N], F32, tag="bias_bc")
                    nc.scalar.dma_start(
                        pv_bc[:],
                        pv_d[:].rearrange("(a n) -> a n", a=1).broadcast_to([128, N]),
                    )
                    nc.scalar.dma_start(
                        bias_bc[:],
                        bias_h[:].rearrange("(a n) -> a n", a=1).broadcast_to([128, N]),
                    )

                    # ---- Phase E: GEMM + dequant epilogue -------------------
                    with tc.tile_pool(name="wqsb", bufs=2) as wqsb, \
                         tc.tile_pool(name="ostage", bufs=4) as ostage:
                        for r in range(C):
                            for ci in range(NCHT):
                                n0 = r * NC + ci * NCH
                                ch8 = wqsb.tile([128, KT, NCH], mybir.dt.int8,
                                                tag="wch8", bufs=1)
                                for k in range(KT):
                                    ceng = nc.scalar if k % 2 == 0 else nc.sync
                                    ceng.dma_start(
                                        ch8[:, k, :],
                                        wq_all_d[r, 128 * k : 128 * (k + 1),
                                                 ci * NCH : (ci + 1) * NCH],
                                    )
                                ch = wqsb.tile([128, KT, NCH], BF16, tag="wch")
                                for k in range(KT):
                                    if k % 2 == 0:
                                        nc.vector.tensor_copy(ch[:, k, :], ch8[:, k, :])
                                    else:
                                        nc.scalar.copy(ch[:, k, :], ch8[:, k, :])
                                for m in range(MT):
                                    ps = psum.tile([128, NCH], F32, tag="ps", bufs=4)
                                    for k in range(KT):
                                        nc.tensor.matmul(
                                            ps[:],
                                            lhsT=xqt[:, k, 128 * m : 128 * (m + 1)],
                                            rhs=ch[:, k, :],
                                            start=(k == 0),
                                            stop=(k == KT - 1),
                                        )
                                    o = ostage.tile([128, NCH], F32, tag="o")
                                    nc.vector.tensor_tensor(
                                        o[:], ps[:], pv_bc[:, n0 : n0 + NCH], op=OP.mult
                                    )
                                    nc.vector.tensor_tensor(
                                        o[:], o[:], bias_bc[:, n0 : n0 + NCH], op=OP.add
                                    )
                                    nc.sync.dma_start(
                                        out_h[128 * m : 128 * (m + 1), n0 : n0 + NCH],
                                        o[:],
                                    )

    nc.finalize()
    return nc


class _Built:
    cache = {}


def _get_built(M, K, N, CAL, n_cores):
    key = (M, K, N, CAL, n_cores)
    if key not in _Built.cache:
        _Built.cache[key] = build_bass(M, K, N, CAL, n_cores)
    return _Built.cache[key]


def make_in_maps(x, weight, bias, calibration, n_cores):
    C = n_cores
    M = x.shape[0]
    N = weight.shape[0]
    CAL = calibration.shape[0]
    MC, NC, CALC = M // C, N // C, CAL // C
    x = np.ascontiguousarray(x, dtype=np.float32)
    weight = np.ascontiguousarray(weight, dtype=np.float32)
    bias = np.ascontiguousarray(bias, dtype=np.float32)
    calibration = np.ascontiguousarray(calibration, dtype=np.float32)
    return [
        {
            "x": x[c * MC : (c + 1) * MC],
            "w": weight[c * NC : (c + 1) * NC],
            "cal": calibration[c * CALC : (c + 1) * CALC],
            "bias": bias,
        }
        for c in range(C)
    ]


def kernel(x, weight, bias, calibration):
    n_cores = 8
    M, K = x.shape
    N = weight.shape[0]
    CAL = calibration.shape[0]
    nc = _get_built(M, K, N, CAL, n_cores)
    in_maps = make_in_maps(x, weight, bias, calibration, n_cores)
    res = run_bass_kernel_spmd(nc, in_maps, list(range(n_cores)))
    out = np.concatenate([res.results[c]["out"] for c in range(n_cores)], axis=0)
    return out.astype(np.float32)
